# revision 27
# baseline (speedup 1.0000x reference)
"""Multi-head attention (causal, interleaved RoPE) on 8 TRN2 NeuronCores.

Sharding: core c = (batch b = c//4, head-group g = c%4). Each core computes
4 heads of one batch fully on-device (QKV proj + RoPE + causal attention +
partial Wo projection); host sums the 4 row-parallel Wo partials per batch.

v2 highlights vs v1:
- RoPE without partition-relayout DMAs: the host permutes W rows so the
  proj PSUM lanes are already in the final qT/kz layout (lane = hh*64 +
  feat, feat = 2f+rr); the rope partner (feat^1) is fetched with a DVE
  stream_shuffle (mask i^1, intra-quadrant), and qT = ps*A + shuffle(ps)*B
  with lane-encoded A=cos, B=+/-sin tiles. Removes 64 dma_starts (each
  costs ~630ns serialized on the shared HWDGE descriptor generator).
- dma_start count minimized (~30 vs 118) and spread across the SP and ACT
  sequencers; outputs are full-row [128,2,512] single DMAs.
- qt=3 normalization is subrange (per 128-q chunk, right after the last PV
  that touches the chunk), and wo3 chains per-t4 through the aux PSUM ring
  so the output tail is ~4us instead of ~12.
"""
import ml_dtypes
import numpy as np

import concourse.bass as bass
import concourse.mybir as mybir
import concourse.tile as tile
from concourse import bacc
from concourse.bass_utils import run_bass_kernel_spmd

f32 = mybir.dt.float32
bf16 = mybir.dt.bfloat16
AF = mybir.ActivationFunctionType

T, D = 2048, 1024
G = 4            # heads per core
NTS = 4          # t-slices of 512
TS = T // NTS    # 512
DCH = D // 128   # 8 contraction chunks
ROPE_BASE = 10000.0
SHUF_MASK = [i ^ 1 for i in range(32)]

_CACHE = {}
DEBUG = False


def _build():
    nc = bacc.Bacc(None, target_bir_lowering=False)
    xt = nc.dram_tensor("xt", [D, T], bf16, kind="ExternalInput")
    wqt = nc.dram_tensor("wqt", [D, 256], bf16, kind="ExternalInput")
    wkt = nc.dram_tensor("wkt", [D, 256], bf16, kind="ExternalInput")
    wvt = nc.dram_tensor("wvt", [D, 256], bf16, kind="ExternalInput")
    wot = nc.dram_tensor("wot", [256, D], bf16, kind="ExternalInput")
    ap_ = nc.dram_tensor("ap", [128, T], bf16, kind="ExternalInput")
    bp_ = nc.dram_tensor("bp", [128, T], bf16, kind="ExternalInput")
    triu = nc.dram_tensor("triu", [128, 128], bf16, kind="ExternalInput")
    outp = nc.dram_tensor("outp", [T, D], bf16, kind="ExternalOutput")
    if DEBUG:
        dqt = nc.dram_tensor("dqt", [128, 2, TS], bf16, kind="ExternalOutput")
        dkz = nc.dram_tensor("dkz", [128, 2, TS], bf16, kind="ExternalOutput")
        dvs = nc.dram_tensor("dvs", [128, 4, G, 65], bf16, kind="ExternalOutput")
        dot = nc.dram_tensor("dot", [128, 2, TS], bf16, kind="ExternalOutput")

    xt_r = xt.rearrange("(dc p) t -> p dc t", p=128)
    wqt_r = wqt.rearrange("(dc p) j -> p dc j", p=128)
    wkt_r = wkt.rearrange("(dc p) j -> p dc j", p=128)
    wvt_r = wvt.rearrange("(dc p) j -> p dc j", p=128)
    wot_r = wot.rearrange("(c p) m -> p c m", p=128)
    outp_r = outp.rearrange("(tt p) m -> p tt m", p=128)

    with tile.TileContext(nc) as tc:
        with (
            tc.tile_pool(name="const", bufs=1) as const,
            tc.tile_pool(name="xtp", bufs=3) as xtp,
            tc.tile_pool(name="ut", bufs=2) as ut,
            tc.tile_pool(name="expp", bufs=4) as expp,
            tc.tile_pool(name="nrm", bufs=2) as nrm,
            tc.tile_pool(name="osb", bufs=4) as osb,
            tc.tile_pool(name="sps", bufs=2, space="PSUM") as sps,
            tc.tile_pool(name="pvp", bufs=1, space="PSUM") as pvp,
            tc.tile_pool(name="aux", bufs=2, space="PSUM") as aux,
        ):
            wq_sb = const.tile([128, DCH, 256], bf16)
            wk_sb = const.tile([128, DCH, 256], bf16)
            wv_sb = const.tile([128, DCH, 256], bf16)
            wo_sb = const.tile([128, 2, D], bf16)
            triu_sb = const.tile([128, 128], bf16)
            a_sb = const.tile([128, T], bf16)
            b_sb = const.tile([128, T], bf16)

            # per-slice persistent tensors (slice-tagged for dependency tracking)
            qTs = [const.tile([128, 2, TS], bf16, name=f"qT{i}", tag=f"qT{i}") for i in range(NTS)]
            kzs = [const.tile([128, 2, TS], bf16, name=f"kz{i}", tag=f"kz{i}") for i in range(NTS)]
            vss = [const.tile([128, 4, G, 65], bf16, name=f"v{i}", tag=f"v{i}") for i in range(NTS)]
            oTs = [const.tile([128, 2, TS], bf16, name=f"oT{i}", tag=f"oT{i}") for i in range(NTS)]
            for i in range(NTS):
                nc.vector.memset(vss[i][:, :, :, 64:65], 1.0)

            xt_sb = {}

            def load_xt(tsi, eng=None):
                xts = xtp.tile([128, DCH, TS], bf16, tag="xts", name="xts")
                (eng or nc.sync).dma_start(xts[:], xt_r[:, :, tsi * TS:(tsi + 1) * TS])
                xt_sb[tsi] = xts

            # DMA order = consumption order. A dma_start's descriptors spread
            # over all 16 DMA queues, so large merged transfers are fine; the
            # scarce resource is the ~630ns/issue shared HWDGE generator, so
            # the issue COUNT is minimized. SP and ACT alternate issues.
            xts0 = xtp.tile([128, DCH, TS], bf16, tag="xts", name="xts")
            xt_sb[0] = xts0
            nc.sync.dma_start(wq_sb[:, 0:2, :], wqt_r[:, 0:2, :])
            nc.scalar.dma_start(xts0[:, 0:2, :], xt_r[:, 0:2, 0:TS])
            nc.sync.dma_start(wq_sb[:, 2:, :], wqt_r[:, 2:, :])
            nc.scalar.dma_start(xts0[:, 2:, :], xt_r[:, 2:, 0:TS])
            nc.sync.dma_start(wk_sb[:], wkt_r[:])
            nc.scalar.dma_start(a_sb[:], ap_[:])
            nc.sync.dma_start(b_sb[:], bp_[:])
            nc.scalar.dma_start(wv_sb[:], wvt_r[:])
            nc.sync.dma_start(triu_sb[:], triu[:])
            load_xt(1, nc.scalar)

            # preload the exp activation table before the attn phase needs it
            # (input is a locally-memset tile so no DMA dependency)
            pre = nrm.tile([1, 8], f32, tag="scr", name="pre")
            nc.vector.memset(pre[:], 0.0)
            scr = nrm.tile([1, 8], f32, tag="scr", name="scr")
            nc.scalar.activation(scr[:], pre[:], AF.Exp)

            # ---------------- filler machinery ----------------
            fillers = []

            def pump(n):
                for _ in range(n):
                    if not fillers:
                        return
                    fillers.pop(0)()

            def proj_qk_units(tsi, w_sb, is_q):
                """QKV projection for one of Wq/Wk: 2 t-chunks x 2 roles x 8
                contraction MMs (N=256) into a 1-bank aux PSUM tile; RoPE on
                DVE via stream_shuffle partner fetch, writing qT/kz direct."""
                units = []
                xtd = xt_sb[tsi]
                dst = qTs[tsi] if is_q else kzs[tsi]
                state = {}

                def unit(c, r, du):
                    if r == 0 and du == 0:
                        state["ps"] = aux.tile([128, 2, 256], f32, tag="aux", name="ps")
                    ps = state["ps"]
                    for d in (2 * du, 2 * du + 1):
                        nc.tensor.matmul(
                            ps[:, r, :],
                            w_sb[:, d, r * 128:(r + 1) * 128],
                            xtd[:, d, c * 256:(c + 1) * 256],
                            start=(d == 0),
                            stop=(d == DCH - 1),
                        )
                    if r == 1 and du == 3:
                        csl = slice(tsi * TS + c * 256, tsi * TS + (c + 1) * 256)
                        rsl = slice(c * 256, (c + 1) * 256)
                        uc = ut.tile([128, 2, 256], bf16, tag="uc", name="uc")
                        xs = ut.tile([128, 2, 256], f32, tag="xs", name="xs")
                        us = ut.tile([128, 2, 256], bf16, tag="us", name="us")
                        nc.vector.tensor_mul(
                            uc[:], ps[:], a_sb[:, None, csl].to_broadcast((128, 2, 256))
                        )
                        nc.vector.stream_shuffle(xs[:], ps[:], SHUF_MASK)
                        nc.vector.tensor_mul(
                            us[:], xs[:], b_sb[:, None, csl].to_broadcast((128, 2, 256))
                        )
                        nc.vector.tensor_add(dst[:, :, rsl], uc[:], us[:])

                for c in range(2):
                    for r in range(2):
                        for du in range(4):
                            units.append(lambda c=c, r=r, du=du: unit(c, r, du))
                return units

            def proj_qk_now(tsi, w_sb, is_q):
                """Immediate (non-filler) proj in du-major wave order: the
                first matmul wave only needs d-chunks 0-1 of w/xt, so compute
                starts as soon as the first small DMAs land. Uses both aux
                ring slots simultaneously — only safe when no other aux user
                interleaves (i.e. outside the filler queue)."""
                xtd = xt_sb[tsi]
                dst = qTs[tsi] if is_q else kzs[tsi]
                ps = {}
                # r-sequential: accumulation groups sharing a PSUM bank (the
                # two r halves of one ps tile) must not interleave on HW — a
                # start=True wipes the whole 2KB zero region. Groups in
                # different banks (c0 vs c1) interleave fine.
                for r in range(2):
                    for du in range(4):
                        for c in range(2):
                            if r == 0 and du == 0 and c not in ps:
                                ps[c] = aux.tile([128, 2, 256], f32, tag="aux", name="ps")
                            for d in (2 * du, 2 * du + 1):
                                nc.tensor.matmul(
                                    ps[c][:, r, :],
                                    w_sb[:, d, r * 128:(r + 1) * 128],
                                    xtd[:, d, c * 256:(c + 1) * 256],
                                    start=(d == 0),
                                    stop=(d == DCH - 1),
                                )
                for c in range(2):
                    csl = slice(tsi * TS + c * 256, tsi * TS + (c + 1) * 256)
                    rsl = slice(c * 256, (c + 1) * 256)
                    uc = ut.tile([128, 2, 256], bf16, tag="uc", name="uc")
                    xs = ut.tile([128, 2, 256], f32, tag="xs", name="xs")
                    us = ut.tile([128, 2, 256], bf16, tag="us", name="us")
                    nc.vector.tensor_mul(
                        uc[:], ps[c][:], a_sb[:, None, csl].to_broadcast((128, 2, 256))
                    )
                    nc.vector.stream_shuffle(xs[:], ps[c][:], SHUF_MASK)
                    nc.vector.tensor_mul(
                        us[:], xs[:], b_sb[:, None, csl].to_broadcast((128, 2, 256))
                    )
                    nc.vector.tensor_add(dst[:, :, rsl], uc[:], us[:])

            def proj_v_units(tsi):
                units = []
                xtd = xt_sb[tsi]
                state = {}

                def unit(st, du):
                    if du == 0:
                        state["psv"] = aux.tile([128, 256], f32, tag="aux", name="psv")
                    psv = state["psv"]
                    for d in (2 * du, 2 * du + 1):
                        nc.tensor.matmul(
                            psv[:],
                            xtd[:, d, st * 128:(st + 1) * 128],
                            wv_sb[:, d, :],
                            start=(d == 0),
                            stop=(d == DCH - 1),
                        )
                    if du == 3:
                        nc.vector.tensor_copy(
                            vss[tsi][:, st, :, 0:64],
                            psv[:].rearrange("p (g dh) -> p g dh", g=G),
                        )

                for st in range(4):
                    for du in range(4):
                        units.append(lambda st=st, du=du: unit(st, du))
                return units

            def wo_units(qt):
                """Wo partial for one finished qt slice: per t4, two aux po
                tiles (mh halves), all 4 matmuls, one merged ob copy + DMA."""
                units = []
                state = {}

                def unit_a(t4):
                    po = [aux.tile([128, TS], f32, tag="aux", name="po") for _ in (0, 1)]
                    state[t4] = po
                    for mh in (0, 1):
                        nc.tensor.matmul(
                            po[mh][:],
                            oTs[qt][:, 0, t4 * 128:(t4 + 1) * 128],
                            wo_sb[:, 0, mh * TS:(mh + 1) * TS],
                            start=True, stop=False,
                        )

                def unit_b(t4):
                    po = state[t4]
                    for mh in (0, 1):
                        nc.tensor.matmul(
                            po[mh][:],
                            oTs[qt][:, 1, t4 * 128:(t4 + 1) * 128],
                            wo_sb[:, 1, mh * TS:(mh + 1) * TS],
                            start=False, stop=True,
                        )
                    ob = osb.tile([128, 2, TS], bf16, tag="ob", name="ob")
                    nc.vector.tensor_copy(ob[:, 0, :], po[0][:])
                    nc.vector.tensor_copy(ob[:, 1, :], po[1][:])
                    nc.sync.dma_start(
                        outp_r[:, qt * 4 + t4, :], ob[:].rearrange("p a b -> p (a b)")
                    )

                for t4 in range(4):
                    units.append(lambda t4=t4: unit_a(t4))
                    units.append(lambda t4=t4: unit_b(t4))
                return units

            # ---------------- attention ----------------
            def norm_chunk(qt, pair, hh, pv, sl, s0_act=False):
                """oT[hh-block, pair, sl] = pv[0:64, sl] / pv[64, sl]."""
                n = sl.stop - sl.start
                s0 = nrm.tile([1, TS], f32, tag="s0", name="s0")
                if s0_act:
                    nc.scalar.copy(s0[:, :n], pv[64:65, sl])
                else:
                    nc.vector.tensor_copy(s0[:, :n], pv[64:65, sl])
                rc = nrm.tile([1, TS], f32, tag="rc", name="rc")
                nc.vector.reciprocal_approx_fast(out=rc[:, :n], in_=s0[:, :n])
                rb = nrm.tile([64, TS], f32, tag="rb", name="rb")
                nc.gpsimd.partition_broadcast(rb[:, :n], rc[:, :n])
                nc.vector.tensor_mul(
                    oTs[qt][hh * 64:(hh + 1) * 64, pair, sl], pv[0:64, sl], rb[:, :n]
                )

            def attn(qt, pump_n, tail_hook=None):
                K = 4 * qt + 4
                for pair in (0, 1):
                    pv = [
                        pvp.tile([65, TS], f32, tag=f"pv{hh}", name=f"pv{hh}")
                        for hh in (0, 1)
                    ]
                    exs = {}
                    offs = {}

                    def S(ko):
                        off = max(0, ko - 4 * qt) * 128
                        offs[ko] = off
                        tko, kin = divmod(ko, 4)
                        ps_s = sps.tile([128, 2, TS], f32, tag="s", name="ps_s")
                        for hh in (0, 1):
                            # K=64 row-tiled pair: tile_position (hh*64, 0)
                            nc.tensor.matmul(
                                ps_s[:, hh, off:],
                                kzs[tko][hh * 64:(hh + 1) * 64, pair, kin * 128:(kin + 1) * 128],
                                qTs[qt][hh * 64:(hh + 1) * 64, pair, off:],
                                start=True,
                                stop=True,
                            )
                        ex = expp.tile([128, 2, TS], bf16, tag="ex", name="ex")
                        nc.scalar.activation(
                            ex[:, :, off:], ps_s[:, :, off:], AF.Exp, scale=0.125
                        )
                        if ko >= 4 * qt:
                            nc.vector.tensor_mul(
                                ex[:, :, off:off + 128],
                                ex[:, :, off:off + 128],
                                triu_sb[:, None, :].to_broadcast((128, 2, 128)),
                            )
                        exs[ko] = ex

                    def PV(ko):
                        off = offs[ko]
                        tko, kin = divmod(ko, 4)
                        for hh in (0, 1):
                            nc.tensor.matmul(
                                pv[hh][:, off:],
                                vss[tko][:, kin, 2 * pair + hh, :],
                                exs[ko][:, hh, off:],
                                start=(ko == 0),
                                stop=(ko == K - 1),
                            )
                        del exs[ko]

                    # normalization is always subrange: the diag PV at
                    # ko = 4qt+t4 finalizes columns [t4*128:(t4+1)*128], so
                    # each chunk normalizes inside the ko loop and pv frees
                    # right after PV(K-1) -> the next pair's PVs don't stall
                    # on a long norm chain (pvp has bufs=1).
                    last = qt == 3 and pair == 1
                    for ko in range(K):
                        S(ko)
                        if ko > 0:
                            PV(ko - 1)
                            if ko - 1 >= 4 * qt:
                                t4 = ko - 1 - 4 * qt
                                # qt3 hook for t4-1 fires one iteration late
                                # so its norm's DVE chain has had time to
                                # land before these matmuls hit the PE head
                                if last and tail_hook and t4 >= 1:
                                    tail_hook(t4 - 1)
                                for hh in (0, 1):
                                    norm_chunk(qt, pair, hh, pv[hh],
                                               slice(t4 * 128, (t4 + 1) * 128),
                                               s0_act=last)
                        pump(pump_n(pair, ko) if callable(pump_n) else pump_n)
                    PV(K - 1)
                    if last and tail_hook:
                        tail_hook(2)
                    for hh in (0, 1):
                        norm_chunk(qt, pair, hh, pv[hh], slice(384, 512),
                                   s0_act=last)
                    if last and tail_hook:
                        tail_hook(3)

            # ---------------- wo3: per-t4 chained through the aux ring ----
            wo3_state = {}

            def wo3_a(t4):
                """hc=0 partials for qt=3 t4 (needs pair-0 norm done)."""
                po = [aux.tile([128, TS], f32, tag="aux", name="po3") for _ in (0, 1)]
                wo3_state[t4] = po
                for mh in (0, 1):
                    nc.tensor.matmul(
                        po[mh][:],
                        oTs[3][:, 0, t4 * 128:(t4 + 1) * 128],
                        wo_sb[:, 0, mh * TS:(mh + 1) * TS],
                        start=True, stop=False,
                    )

            def wo3_b(t4):
                """hc=1 + merged output (needs pair-1 subrange norm of t4)."""
                po = wo3_state[t4]
                for mh in (0, 1):
                    nc.tensor.matmul(
                        po[mh][:],
                        oTs[3][:, 1, t4 * 128:(t4 + 1) * 128],
                        wo_sb[:, 1, mh * TS:(mh + 1) * TS],
                        start=False, stop=True,
                    )
                ob = osb.tile([128, 2, TS], bf16, tag="ob", name="ob")
                if t4 >= 2:
                    # ACT is done with exp by now; split the cast across engines
                    nc.scalar.copy(ob[:, 0, :], po[0][:])
                    nc.vector.tensor_copy(ob[:, 1, :], po[1][:])
                else:
                    nc.vector.tensor_copy(ob[:, 0, :], po[0][:])
                    nc.vector.tensor_copy(ob[:, 1, :], po[1][:])
                eng = nc.scalar if t4 % 2 == 0 else nc.sync
                eng.dma_start(
                    outp_r[:, 12 + t4, :], ob[:].rearrange("p a b -> p (a b)")
                )

            def wo3_tail(t4):
                wo3_b(t4)
                if t4 + 1 <= 3:
                    wo3_a(t4 + 1)

            # ---------------- schedule ----------------
            import os as _os2
            if _os2.environ.get("DUMAJOR", "1") == "1":
                proj_qk_now(0, wq_sb, True)
                proj_qk_now(0, wk_sb, False)
            else:
                for u in proj_qk_units(0, wq_sb, True):
                    u()
                for u in proj_qk_units(0, wk_sb, False):
                    u()

            # V(0) rides the filler queue: its matmuls precede (in queue
            # order) the PV(ko) that consumes each st-chunk, so attn(0)'s
            # S/exp start earlier while wv/xt still stream in
            fillers += proj_v_units(0)
            fillers += proj_qk_units(1, wq_sb, True)
            fillers += proj_qk_units(1, wk_sb, False)
            fillers += proj_v_units(1)
            # xt2 issued before attn(0): the sync sequencer is free and the
            # 8 chunk transfers drain during attn(0) (xtp bufs=3 so no WAR
            # wait on xts0)
            load_xt(2)
            attn(0, 8)
            while fillers:          # drain proj(1) before attn(1) needs it
                fillers.pop(0)()
            nc.sync.dma_start(wo_sb[:], wot_r)
            fillers += proj_qk_units(2, wq_sb, True)
            fillers += proj_qk_units(2, wk_sb, False)
            fillers += proj_v_units(2)
            attn(1, 3)
            load_xt(3)
            fillers += wo_units(0)
            fillers += proj_qk_units(3, wq_sb, True)
            fillers += proj_qk_units(3, wk_sb, False)
            attn(2, 2)
            fillers += proj_v_units(3)
            fillers += wo_units(1)
            fillers += wo_units(2)
            # wo3: hc0 for t4=0 rides the fillers (runs after pair-0 norm,
            # during late attn(3)); each t4's hc1+output chains right after
            # the pair-1 subrange norm of that t4, and schedules hc0 of the
            # next t4 through the freed aux ring.
            fillers += [lambda: wo3_a(0)]
            attn(3, lambda pair, ko: 2 if ko < 8 else 1, tail_hook=wo3_tail)
            while fillers:
                fillers.pop(0)()
            if DEBUG:
                import os as _os
                _sl = int(_os.environ.get("DBG_SLICE", "0"))
                nc.sync.dma_start(dqt[:], qTs[_sl][:])
                nc.sync.dma_start(dkz[:], kzs[_sl][:])
                nc.sync.dma_start(dvs[:], vss[_sl][:])
                nc.sync.dma_start(dot[:], oTs[_sl][:])
    nc.compile()
    return nc


def _get_nc():
    if "nc" not in _CACHE:
        _CACHE["nc"] = _build()
    return _CACHE["nc"]


def _host_inputs(x, Wq, Wk, Wv, Wo):
    """Build per-core input dicts (host-side sharding / layout prep).

    Lane layout for qT/kz (and the W row permute): position j = r*128 + p,
    p = hh*64 + feat, feat = 2f + rr; head local h = 2r + hh, W row =
    (g*4 + h)*64 + feat. A[p,t] = cos(t*invfreq[feat//2]); B[p,t] =
    -sin if feat even else +sin.
    """
    jj = np.arange(256)
    r = jj // 128
    hh = (jj % 128) // 64
    feat = jj % 64
    h = 2 * r + hh
    inv_freq = 1.0 / (ROPE_BASE ** (np.arange(0, 64, 2, dtype=np.float64) / 64.0))
    t = np.arange(T, dtype=np.float64)
    lane_f = (np.arange(128) % 64) // 2
    lane_rr = np.arange(128) % 2
    ang = t[None, :] * inv_freq[lane_f][:, None]   # [128, T]
    ap = np.cos(ang).astype(ml_dtypes.bfloat16)
    bp = (np.where(lane_rr[:, None] == 0, -1.0, 1.0) * np.sin(ang)).astype(ml_dtypes.bfloat16)
    triu = (np.arange(128)[None, :] >= np.arange(128)[:, None]).astype(ml_dtypes.bfloat16)

    in_maps = []
    for core in range(8):
        b, g = divmod(core, 4)
        jsel = (g * 4 + h) * 64 + feat
        in_maps.append({
            "xt": np.ascontiguousarray(x[b].T).astype(ml_dtypes.bfloat16),
            "wqt": np.ascontiguousarray(Wq[jsel, :].T).astype(ml_dtypes.bfloat16),
            "wkt": np.ascontiguousarray(Wk[jsel, :].T).astype(ml_dtypes.bfloat16),
            "wvt": np.ascontiguousarray(Wv[g * 256:(g + 1) * 256, :].T).astype(ml_dtypes.bfloat16),
            "wot": np.ascontiguousarray(Wo[:, g * 256:(g + 1) * 256].T).astype(ml_dtypes.bfloat16),
            "ap": ap,
            "bp": bp,
            "triu": triu,
        })
    return in_maps


def run(x, Wq, Wk, Wv, Wo, trace=False):
    nc = _get_nc()
    in_maps = _host_inputs(x, Wq, Wk, Wv, Wo)
    res = run_bass_kernel_spmd(nc, in_maps, core_ids=list(range(8)), trace=trace)
    out = np.zeros((2, T, D), dtype=np.float64)
    for core in range(8):
        out[core // 4] += res.results[core]["outp"].astype(np.float64)
    return out.astype(np.float32), res


def kernel(x=None, mask=None, Wq=None, Wk=None, Wv=None, Wo=None, **_ignored):
    x = np.asarray(x, dtype=np.float32)
    Wq = np.asarray(Wq, dtype=np.float32)
    Wk = np.asarray(Wk, dtype=np.float32)
    Wv = np.asarray(Wv, dtype=np.float32)
    Wo = np.asarray(Wo, dtype=np.float32)
    out, _ = run(x, Wq, Wk, Wv, Wo, trace=False)
    return out


# revision 28
# speedup vs baseline: 1.1253x; 1.1253x over previous
"""Multi-head attention (causal, interleaved RoPE) on 8 TRN2 NeuronCores.

Sharding: core c = (batch b = c//4, head-group g = c%4). Each core computes
4 heads of one batch fully on-device (QKV proj + RoPE + causal attention +
partial Wo projection); host sums the 4 row-parallel Wo partials per batch.

Single pipelined schedule: attention S^T runs as two concurrent K=64
row-tiled matmuls (tile_position via base partitions 0/64), PV uses the
[V|1] ones-column trick for softmax sums, and all matmul operands are bf16.
proj/wo matmuls are drip-fed as "filler" units into the per-iteration PE
bubbles left by the exp latency.

RoPE without partition-relayout DMAs: the host orders W rows so the proj
PSUM lanes are already in the final qT/kz layout (lane = hh*64 + feat,
feat = 2f+rr); the rope partner (feat^1) comes from a DVE stream_shuffle
(mask i^1, intra-quadrant) and qT = ps*A + shuffle(ps)*B with lane-encoded
A=cos, B=+/-sin tiles. This removes 64 dma_starts (each costs ~630ns
serialized on the shared HWDGE descriptor generator, which was the real
reason attention could not start before ~37us).
PSUM budget: sps 2x2 banks + pv 2 banks + shared aux ring 2 banks = 8.
"""
import ml_dtypes
import numpy as np

import concourse.bass as bass
import concourse.mybir as mybir
import concourse.tile as tile
from concourse import bacc
from concourse.bass_utils import run_bass_kernel_spmd

f32 = mybir.dt.float32
bf16 = mybir.dt.bfloat16
AF = mybir.ActivationFunctionType

T, D = 2048, 1024
G = 4            # heads per core
NTS = 4          # t-slices of 512
TS = T // NTS    # 512
DCH = D // 128   # 8 contraction chunks
ROPE_BASE = 10000.0
SHUF_MASK = [i ^ 1 for i in range(32)]

_CACHE = {}


def _build():
    nc = bacc.Bacc(None, target_bir_lowering=False)
    xt = nc.dram_tensor("xt", [D, T], bf16, kind="ExternalInput")
    wqt = nc.dram_tensor("wqt", [D, 256], bf16, kind="ExternalInput")
    wkt = nc.dram_tensor("wkt", [D, 256], bf16, kind="ExternalInput")
    wvt = nc.dram_tensor("wvt", [D, 256], bf16, kind="ExternalInput")
    wot = nc.dram_tensor("wot", [256, D], bf16, kind="ExternalInput")
    ap_ = nc.dram_tensor("ap", [128, T], bf16, kind="ExternalInput")
    bp_ = nc.dram_tensor("bp", [128, T], bf16, kind="ExternalInput")
    triu = nc.dram_tensor("triu", [128, 128], bf16, kind="ExternalInput")
    outp = nc.dram_tensor("outp", [T, D], bf16, kind="ExternalOutput")

    xt_r = xt.rearrange("(dc p) t -> p dc t", p=128)
    wqt_r = wqt.rearrange("(dc p) j -> p dc j", p=128)
    wkt_r = wkt.rearrange("(dc p) j -> p dc j", p=128)
    wvt_r = wvt.rearrange("(dc p) j -> p dc j", p=128)
    wot_r = wot.rearrange("(c p) m -> p c m", p=128)
    outp_r = outp.rearrange("(tt p) m -> p tt m", p=128)

    with tile.TileContext(nc) as tc:
        with (
            tc.tile_pool(name="const", bufs=1) as const,
            tc.tile_pool(name="xtp", bufs=3) as xtp,
            tc.tile_pool(name="ut", bufs=2) as ut,
            tc.tile_pool(name="expp", bufs=4) as expp,
            tc.tile_pool(name="nrm", bufs=2) as nrm,
            tc.tile_pool(name="osb", bufs=4) as osb,
            tc.tile_pool(name="sps", bufs=2, space="PSUM") as sps,
            tc.tile_pool(name="pvp", bufs=1, space="PSUM") as pvp,
            tc.tile_pool(name="aux", bufs=2, space="PSUM") as aux,
        ):
            wq_sb = const.tile([128, DCH, 256], bf16)
            wk_sb = const.tile([128, DCH, 256], bf16)
            wv_sb = const.tile([128, DCH, 256], bf16)
            wo_sb = const.tile([128, 2, D], bf16)
            triu_sb = const.tile([128, 128], bf16)
            a_sb = const.tile([128, T], bf16)
            b_sb = const.tile([128, T], bf16)

            # per-slice persistent tensors (slice-tagged for dependency tracking)
            qTs = [const.tile([128, 2, TS], bf16, name=f"qT{i}", tag=f"qT{i}") for i in range(NTS)]
            kzs = [const.tile([128, 2, TS], bf16, name=f"kz{i}", tag=f"kz{i}") for i in range(NTS)]
            vss = [const.tile([128, 4, G, 65], bf16, name=f"v{i}", tag=f"v{i}") for i in range(NTS)]
            oTs = [const.tile([128, 2, TS], bf16, name=f"oT{i}", tag=f"oT{i}") for i in range(NTS)]
            for i in range(NTS):
                nc.vector.memset(vss[i][:, :, :, 64:65], 1.0)

            xt_sb = {}

            def load_xt(tsi, eng=None):
                xts = xtp.tile([128, DCH, TS], bf16, tag="xts", name="xts")
                (eng or nc.sync).dma_start(xts[:], xt_r[:, :, tsi * TS:(tsi + 1) * TS])
                xt_sb[tsi] = xts

            # DMA order = consumption order. One dma_start's descriptors
            # spread over all 16 DMA queues, so merged transfers are fine;
            # the scarce resource is the ~630ns/issue shared HWDGE
            # generator, so the issue COUNT is minimized and split between
            # the SP and ACT sequencers (both idle here).
            nc.sync.dma_start(wq_sb[:, 0:2, :], wqt_r[:, 0:2, :])
            xts0 = xtp.tile([128, DCH, TS], bf16, tag="xts", name="xts")
            nc.scalar.dma_start(xts0[:, 0:2, :], xt_r[:, 0:2, 0:TS])
            nc.sync.dma_start(wq_sb[:, 2:, :], wqt_r[:, 2:, :])
            nc.scalar.dma_start(xts0[:, 2:, :], xt_r[:, 2:, 0:TS])
            xt_sb[0] = xts0
            nc.sync.dma_start(wk_sb[:], wkt_r[:])
            nc.scalar.dma_start(a_sb[:], ap_[:])
            nc.sync.dma_start(b_sb[:], bp_[:])
            nc.scalar.dma_start(wv_sb[:], wvt_r[:])
            nc.sync.dma_start(triu_sb[:], triu[:])
            load_xt(1, nc.scalar)

            # preload the exp activation table before the attn phase needs it
            # (input is a locally-memset tile so no DMA dependency)
            pre = nrm.tile([1, 8], f32, tag="scr", name="pre")
            nc.vector.memset(pre[:], 0.0)
            scr = nrm.tile([1, 8], f32, tag="scr", name="scr")
            nc.scalar.activation(scr[:], pre[:], AF.Exp)

            # ---------------- filler machinery ----------------
            fillers = []

            def pump(n):
                for _ in range(n):
                    if not fillers:
                        return
                    fillers.pop(0)()

            def proj_qk_units(tsi, w_sb, is_q):
                """QKV projection for one of Wq/Wk: 2 t-chunks x 2 roles x 8
                contraction MMs (N=256) into a 1-bank aux PSUM tile, RoPE'd on
                DVE, relaid to qT/kz via partition-shuffling DMAs."""
                units = []
                xtd = xt_sb[tsi]
                dst = qTs[tsi] if is_q else kzs[tsi]
                state = {}

                def unit(c, r, du):
                    if r == 0 and du == 0:
                        state["ps"] = aux.tile([128, 2, 256], f32, tag="aux", name="ps")
                    ps = state["ps"]
                    for d in (2 * du, 2 * du + 1):
                        nc.tensor.matmul(
                            ps[:, r, :],
                            w_sb[:, d, r * 128:(r + 1) * 128],
                            xtd[:, d, c * 256:(c + 1) * 256],
                            start=(d == 0),
                            stop=(d == DCH - 1),
                        )
                    if r == 1 and du == 3:
                        csl = slice(tsi * TS + c * 256, tsi * TS + (c + 1) * 256)
                        rsl = slice(c * 256, (c + 1) * 256)
                        uc = ut.tile([128, 2, 256], bf16, tag="uc", name="uc")
                        xs = ut.tile([128, 2, 256], f32, tag="xs", name="xs")
                        us = ut.tile([128, 2, 256], bf16, tag="us", name="us")
                        nc.vector.tensor_mul(
                            uc[:], ps[:], a_sb[:, None, csl].to_broadcast((128, 2, 256))
                        )
                        nc.vector.stream_shuffle(xs[:], ps[:], SHUF_MASK)
                        nc.vector.tensor_mul(
                            us[:], xs[:], b_sb[:, None, csl].to_broadcast((128, 2, 256))
                        )
                        nc.vector.tensor_add(dst[:, :, rsl], uc[:], us[:])

                for c in range(2):
                    for r in range(2):
                        for du in range(4):
                            units.append(lambda c=c, r=r, du=du: unit(c, r, du))
                return units

            def proj_v_units(tsi):
                units = []
                xtd = xt_sb[tsi]
                state = {}

                def unit(st, du):
                    if du == 0:
                        state["psv"] = aux.tile([128, 256], f32, tag="aux", name="psv")
                    psv = state["psv"]
                    for d in (2 * du, 2 * du + 1):
                        nc.tensor.matmul(
                            psv[:],
                            xtd[:, d, st * 128:(st + 1) * 128],
                            wv_sb[:, d, :],
                            start=(d == 0),
                            stop=(d == DCH - 1),
                        )
                    if du == 3:
                        nc.vector.tensor_copy(
                            vss[tsi][:, st, :, 0:64],
                            psv[:].rearrange("p (g dh) -> p g dh", g=G),
                        )

                for st in range(4):
                    for du in range(4):
                        units.append(lambda st=st, du=du: unit(st, du))
                return units

            def wo_units(qt, on_scalar=False):
                units = []

                def unit(t4, mh):
                    po = aux.tile([128, TS], f32, tag="aux", name="po")
                    for hc in (0, 1):
                        nc.tensor.matmul(
                            po[:],
                            oTs[qt][:, hc, t4 * 128:(t4 + 1) * 128],
                            wo_sb[:, hc, mh * TS:(mh + 1) * TS],
                            start=(hc == 0),
                            stop=(hc == 1),
                        )
                    ob = osb.tile([128, TS], bf16, tag="ob", name="ob")
                    if on_scalar and (t4 + mh) % 2 == 0:
                        nc.scalar.copy(ob[:], po[:])
                    else:
                        nc.vector.tensor_copy(ob[:], po[:])
                    nc.sync.dma_start(outp_r[:, qt * 4 + t4, mh * TS:(mh + 1) * TS], ob[:])

                for t4 in range(4):
                    for mh in (0, 1):
                        units.append(lambda t4=t4, mh=mh: unit(t4, mh))
                return units

            # ---------------- attention ----------------
            def attn(qt, pump_n):
                K = 4 * qt + 4
                for pair in (0, 1):
                    pv = [
                        pvp.tile([65, TS], f32, tag=f"pv{hh}", name=f"pv{hh}")
                        for hh in (0, 1)
                    ]
                    exs = {}
                    offs = {}

                    def S(ko):
                        off = max(0, ko - 4 * qt) * 128
                        offs[ko] = off
                        tko, kin = divmod(ko, 4)
                        ps_s = sps.tile([128, 2, TS], f32, tag="s", name="ps_s")
                        for hh in (0, 1):
                            # K=64 row-tiled pair: tile_position (hh*64, 0)
                            nc.tensor.matmul(
                                ps_s[:, hh, off:],
                                kzs[tko][hh * 64:(hh + 1) * 64, pair, kin * 128:(kin + 1) * 128],
                                qTs[qt][hh * 64:(hh + 1) * 64, pair, off:],
                                start=True,
                                stop=True,
                            )
                        ex = expp.tile([128, 2, TS], bf16, tag="ex", name="ex")
                        nc.scalar.activation(
                            ex[:, :, off:], ps_s[:, :, off:], AF.Exp, scale=0.125
                        )
                        if ko >= 4 * qt:
                            nc.vector.tensor_mul(
                                ex[:, :, off:off + 128],
                                ex[:, :, off:off + 128],
                                triu_sb[:, None, :].to_broadcast((128, 2, 128)),
                            )
                        exs[ko] = ex

                    def PV(ko):
                        off = offs[ko]
                        tko, kin = divmod(ko, 4)
                        for hh in (0, 1):
                            nc.tensor.matmul(
                                pv[hh][:, off:],
                                vss[tko][:, kin, 2 * pair + hh, :],
                                exs[ko][:, hh, off:],
                                start=(ko == 0),
                                stop=(ko == K - 1),
                            )
                        del exs[ko]

                    for ko in range(K):
                        S(ko)
                        if ko > 0:
                            PV(ko - 1)
                        pump(pump_n(pair, ko) if callable(pump_n) else pump_n)
                    PV(K - 1)

                    for hh in (0, 1):
                        s0 = nrm.tile([1, TS], f32, tag="s0", name="s0")
                        if qt == 3 and pair == 1:
                            # ScalarE is idle after the final exp; keep DVE
                            # free for the recip/mul tail chain
                            nc.scalar.copy(s0[:], pv[hh][64:65, :])
                        else:
                            nc.vector.tensor_copy(s0[:], pv[hh][64:65, :])
                        rc = nrm.tile([1, TS], f32, tag="rc", name="rc")
                        nc.vector.reciprocal_approx_fast(out=rc[:], in_=s0[:])
                        rb = nrm.tile([64, TS], f32, tag="rb", name="rb")
                        nc.gpsimd.partition_broadcast(rb[:], rc[:])
                        nc.vector.tensor_mul(
                            oTs[qt][hh * 64:(hh + 1) * 64, pair, :], pv[hh][0:64, :], rb[:]
                        )

            # ---------------- schedule ----------------
            for u in proj_qk_units(0, wq_sb, True):
                u()
            for u in proj_qk_units(0, wk_sb, False):
                u()

            # V(0) rides the filler queue: its matmuls precede (in queue
            # order) the PV(ko) that consumes each st-chunk, so attn(0)'s
            # S/exp start ~6us earlier while wv/xt still stream in
            fillers += proj_v_units(0)
            fillers += proj_qk_units(1, wq_sb, True)
            fillers += proj_qk_units(1, wk_sb, False)
            fillers += proj_v_units(1)
            load_xt(2)              # sync is idle; xtp bufs=3 -> no WAR wait
            attn(0, 6)
            while fillers:          # drain proj(1) before attn(1) needs it
                fillers.pop(0)()
            nc.sync.dma_start(wo_sb[:], wot_r)
            fillers += proj_qk_units(2, wq_sb, True)
            fillers += proj_qk_units(2, wk_sb, False)
            fillers += proj_v_units(2)
            attn(1, 3)
            load_xt(3)
            fillers += wo_units(0)
            fillers += proj_qk_units(3, wq_sb, True)
            fillers += proj_qk_units(3, wk_sb, False)
            attn(2, 2)
            fillers += proj_v_units(3)
            fillers += wo_units(1)
            fillers += wo_units(2)
            # wo(3): output halves spread across all free PSUM slots;
            # t4=2's hc0 matmuls ride the filler queue (run after pair-0's
            # norm, during late attn(3)); the rest chase the final exp/norm.
            wo3_tiles = {}

            def wo3_mm(key, t4, mh, hc):
                po = wo3_tiles[key]
                sel = po[:, mh, :] if len(po.shape) == 3 else po[:]
                nc.tensor.matmul(
                    sel,
                    oTs[3][:, hc, t4 * 128:(t4 + 1) * 128],
                    wo_sb[:, hc, mh * TS:(mh + 1) * TS],
                    start=(hc == 0),
                    stop=(hc == 1),
                )

            def wo3_aux_unit(mh):
                def u(mh=mh):
                    wo3_tiles[("a", mh)] = aux.tile([128, TS], f32, tag="aux", name="poa")
                    wo3_mm(("a", mh), 2, mh, 0)
                return u

            fillers += [wo3_aux_unit(0), wo3_aux_unit(1)]
            attn(3, lambda pair, ko: 2 if ko < 8 else 1)
            while fillers:
                fillers.pop(0)()

            wo3_tiles[("s", 0)] = sps.tile([128, 2, TS], f32, tag="s", name="po2")
            wo3_mm(("s", 0), 0, 0, 0)
            wo3_mm(("s", 0), 0, 1, 0)
            wo3_tiles[("s", 1)] = sps.tile([128, 2, TS], f32, tag="s", name="po2")
            wo3_mm(("s", 1), 1, 0, 0)
            wo3_mm(("s", 1), 1, 1, 0)
            wo3_tiles[("p", 0)] = pvp.tile([128, TS], f32, tag="pv0", name="pop")
            wo3_tiles[("p", 1)] = pvp.tile([128, TS], f32, tag="pv1", name="pop")
            wo3_mm(("p", 0), 3, 0, 0)
            wo3_mm(("p", 1), 3, 1, 0)
            def wo3_out(i, key, t4, mh):
                po = wo3_tiles[key]
                if mh is None:
                    ob = osb.tile([128, 2, TS], bf16, tag="ob2", name="ob2")
                else:
                    ob = osb.tile([128, TS], bf16, tag="ob", name="ob")
                if i % 2 == 0:
                    nc.scalar.copy(ob[:], po[:])
                else:
                    nc.vector.tensor_copy(ob[:], po[:])
                if mh is None:
                    nc.sync.dma_start(
                        outp_r[:, 12 + t4, :], ob[:].rearrange("p a b -> p (a b)")
                    )
                else:
                    nc.sync.dma_start(outp_r[:, 12 + t4, mh * TS:(mh + 1) * TS], ob[:])

            wo3_mm(("a", 0), 2, 0, 1)
            wo3_out(0, ("a", 0), 2, 0)
            wo3_mm(("a", 1), 2, 1, 1)
            wo3_out(1, ("a", 1), 2, 1)
            wo3_mm(("s", 0), 0, 0, 1)
            wo3_mm(("s", 0), 0, 1, 1)
            wo3_out(2, ("s", 0), 0, None)
            wo3_mm(("s", 1), 1, 0, 1)
            wo3_mm(("s", 1), 1, 1, 1)
            wo3_out(3, ("s", 1), 1, None)
            wo3_mm(("p", 0), 3, 0, 1)
            wo3_out(4, ("p", 0), 3, 0)
            wo3_mm(("p", 1), 3, 1, 1)
            wo3_out(5, ("p", 1), 3, 1)
    nc.compile()
    return nc


def _get_nc():
    if "nc" not in _CACHE:
        _CACHE["nc"] = _build()
    return _CACHE["nc"]


def _host_inputs(x, Wq, Wk, Wv, Wo):
    """Build per-core input dicts (host-side sharding / layout prep)."""
    # lane layout: position j = r*128 + p, p = hh*64 + feat, feat = 2f+rr;
    # head local h = 2r + hh, W row = (g*4 + h)*64 + feat (i.e. head-major
    # natural order). A[p,t] = cos(t*invfreq[feat//2]); B[p,t] = -sin for
    # even feat (rr=0) else +sin.
    inv_freq = 1.0 / (ROPE_BASE ** (np.arange(0, 64, 2, dtype=np.float64) / 64.0))
    t = np.arange(T, dtype=np.float64)
    lane_f = (np.arange(128) % 64) // 2
    lane_rr = np.arange(128) % 2
    ang = t[None, :] * inv_freq[lane_f][:, None]   # [128, T]
    ap = np.cos(ang).astype(ml_dtypes.bfloat16)
    bp = (np.where(lane_rr[:, None] == 0, -1.0, 1.0) * np.sin(ang)).astype(ml_dtypes.bfloat16)
    triu = (np.arange(128)[None, :] >= np.arange(128)[:, None]).astype(ml_dtypes.bfloat16)

    in_maps = []
    for core in range(8):
        b, g = divmod(core, 4)
        in_maps.append({
            "xt": np.ascontiguousarray(x[b].T).astype(ml_dtypes.bfloat16),
            "wqt": np.ascontiguousarray(Wq[g * 256:(g + 1) * 256, :].T).astype(ml_dtypes.bfloat16),
            "wkt": np.ascontiguousarray(Wk[g * 256:(g + 1) * 256, :].T).astype(ml_dtypes.bfloat16),
            "wvt": np.ascontiguousarray(Wv[g * 256:(g + 1) * 256, :].T).astype(ml_dtypes.bfloat16),
            "wot": np.ascontiguousarray(Wo[:, g * 256:(g + 1) * 256].T).astype(ml_dtypes.bfloat16),
            "ap": ap,
            "bp": bp,
            "triu": triu,
        })
    return in_maps


def run(x, Wq, Wk, Wv, Wo, trace=False):
    nc = _get_nc()
    in_maps = _host_inputs(x, Wq, Wk, Wv, Wo)
    res = run_bass_kernel_spmd(nc, in_maps, core_ids=list(range(8)), trace=trace)
    out = np.zeros((2, T, D), dtype=np.float64)
    for core in range(8):
        out[core // 4] += res.results[core]["outp"].astype(np.float64)
    return out.astype(np.float32), res


def kernel(x=None, mask=None, Wq=None, Wk=None, Wv=None, Wo=None, **_ignored):
    x = np.asarray(x, dtype=np.float32)
    Wq = np.asarray(Wq, dtype=np.float32)
    Wk = np.asarray(Wk, dtype=np.float32)
    Wv = np.asarray(Wv, dtype=np.float32)
    Wo = np.asarray(Wo, dtype=np.float32)
    out, _ = run(x, Wq, Wk, Wv, Wo, trace=False)
    return out

